# revision 15
# baseline (speedup 1.0000x reference)
"""Multi-head causal self-attention (torch nn.MultiheadAttention semantics)
on 8 Trainium2 NeuronCores.

Problem: x [2, 2048, 1024], 16 heads, head dim 64, fp32, causal, p_drop=0.

Sharding: 2 batch groups x 4-way head tensor-parallel.
  core c: batch b = c // 4, heads [lane*4, lane*4+4) with lane = c % 4.
Each core computes q/k/v projections for its 4 heads, flash-style causal
attention (S^T score layout, no-max softmax — scores are O(1) here), and its
partial out-projection. The host sums the 4 partials per batch and adds b_out.

v5 (trace-driven, from v3 @ 202us, v4b @ 188.5us):
  - Only sync (SP) and scalar (ACT) queues have fast hardware DMA (HWDGE,
    ~250GB/s multi-ring); gpsimd dma_start is a slow single-ring software
    path (~50GB/s). Critical prefix (wqk[0:2], x[0:2] of span 0, wv) issues
    on the scalar queue (ACT idle until first exp ~14us), the bulk on sync,
    only the tiny biases on gpsimd.
  - reciprocal -> reciprocal_approx_fast (~5x): the 3.35us iterative divide
    head-of-line blocked the PSUM-freeing bias-adds at span boundaries.
  - No oraw copies: normalize muls (DVE) read the PV PSUM accumulators
    directly (GPSIMD cannot touch PSUM; ACT copies were on the exp pacer).
  - W_out/OT bf16 (same PE rate at 512 free dim), b_v broadcast on chip.
  - LAG=3: PV trails scores more so PE rarely waits on the exp stream.
  - Fill chains spread evenly across steps (ACT exp pace is ~2.1us/step;
    clustered fills let the ps PSUM ring drain and stall later scores).
  - Tail: span-3 PV columns finalize progressively (cols q*128:(q+1)*128
    are final after k-block 12+q), so each 128-col quarter's den/normalize/
    out-proj/DMA chain is emitted right after that PV step, overlapping the
    remaining PV work; last out DMAs ride the fast scalar/sync queues.
"""

import os
from contextlib import ExitStack

import ml_dtypes
import numpy as np

import concourse.bass as bass
import concourse.tile as tile
from concourse import bacc, mybir
from concourse.bass_utils import run_bass_kernel_spmd

F32 = mybir.dt.float32
BF16 = mybir.dt.bfloat16
AF = mybir.ActivationFunctionType

B = 2
S = 2048
DM = 1024
N_HEADS = 16
DH = 64
N_CORES = 8
CPG = 4  # cores per group (tensor-parallel width over heads)
HPC = N_HEADS // CPG  # heads per core
DQ = HPC * DH
SPAN = 512
SB = 128
NDM = DM // 128
NSPAN = S // SPAN
NSB = S // SB
SBS = SPAN // SB
NQK = 2 * DQ // 128
NHD = DQ // 128
VW = DH + 1
OW = 512
NOUT = DM // OW
LAG = 3  # PV trails scores by this many sk blocks


def _declare_io(nc):
    t = {}
    t["xT"] = nc.dram_tensor("xT", [DM, S], BF16, kind="ExternalInput").ap()
    t["wqkT"] = nc.dram_tensor("wqkT", [DM, 2 * DQ], BF16, kind="ExternalInput").ap()
    t["wvT"] = nc.dram_tensor("wvT", [DM, DQ], BF16, kind="ExternalInput").ap()
    t["woT"] = nc.dram_tensor("woT", [DQ, DM], BF16, kind="ExternalInput").ap()
    t["bqk"] = nc.dram_tensor("bqk", [2 * DQ, 1], F32, kind="ExternalInput").ap()
    t["bv"] = nc.dram_tensor("bv", [1, DQ], F32, kind="ExternalInput").ap()
    t["out"] = nc.dram_tensor("out", [S, DM], BF16, kind="ExternalOutput").ap()
    return t


def _build(ctx: ExitStack, tc: tile.TileContext, io: dict):
    nc = tc.nc

    const = ctx.enter_context(tc.tile_pool(name="const", bufs=1))
    work = ctx.enter_context(tc.tile_pool(name="work", bufs=1))
    psum = ctx.enter_context(tc.tile_pool(name="psum", bufs=1, space="PSUM"))

    # ---- persistent tiles ----
    xTb = const.tile([128, NDM, S], BF16, name="xTb")
    wqkb = const.tile([128, NDM, 2 * DQ], BF16, name="wqkb")
    wvb = const.tile([128, NDM, DQ], BF16, name="wvb")
    wob = const.tile([128, NHD, DM], BF16, name="wob")
    bqkb = const.tile([128, NQK], F32, name="bqkb")
    bqk = [bqkb[:, b : b + 1] for b in range(NQK)]
    bvs = const.tile([1, DQ], F32, name="bvs")
    bv = const.tile([128, DQ], F32, name="bv")
    qkT = [const.tile([128, S], BF16, name=f"qkT{b}") for b in range(NQK)]
    vp = [const.tile([128, HPC * VW], BF16, name=f"vp{sb}") for sb in range(NSB)]
    OT = [const.tile([128, S], BF16, name=f"OT{c}") for c in range(NHD)]

    # ---- input DMAs (fast HWDGE queues: sync + scalar; gpsimd is slow).
    # Two balanced streams: first-needed piece leads each queue so the
    # ph1(0) chains can chase c-chunk arrivals. ----
    xTd = io["xT"].rearrange("(c p) s -> p c s", p=128)
    wqkd = io["wqkT"].rearrange("(c p) w -> p c w", p=128)
    nc.scalar.dma_start(wqkb[:, 0:2, :], wqkd[:, 0:2, :])
    nc.scalar.dma_start(wqkb[:, 2:NDM, :], wqkd[:, 2:NDM, :])
    nc.scalar.dma_start(wvb[:], io["wvT"].rearrange("(c p) w -> p c w", p=128))
    nc.sync.dma_start(xTb[:, 0:2, 0:SPAN], xTd[:, 0:2, 0:SPAN])
    nc.sync.dma_start(xTb[:, 2:NDM, 0:SPAN], xTd[:, 2:NDM, 0:SPAN])
    for sp in range(1, NSPAN):
        nc.sync.dma_start(
            xTb[:, :, sp * SPAN : (sp + 1) * SPAN],
            xTd[:, :, sp * SPAN : (sp + 1) * SPAN],
        )
    nc.sync.dma_start(wob[:], io["woT"].rearrange("(c p) w -> p c w", p=128))
    nc.gpsimd.dma_start(bqkb[:], io["bqk"].rearrange("(b p) o -> p (b o)", p=128))
    nc.gpsimd.dma_start(bvs[:], io["bv"][:])

    # ---- constants: bv broadcast, vp ones-columns, replicated causal tri ----
    nc.gpsimd.partition_broadcast(bv[:], bvs[0:1, :])
    for sb in range(NSB):
        nc.gpsimd.memset(vp[sb][:, DH : HPC * VW : VW], 1.0)
    # tri4 = 4 side-by-side copies of tri[r, c] = (c - r >= 0)
    tri4 = const.tile([128, 4 * 128], BF16, name="tri4")
    nc.gpsimd.memset(tri4[:], 1.0)
    for k in range(4):
        nc.gpsimd.affine_select(
            out=tri4[:, k * 128 : (k + 1) * 128],
            in_=tri4[:, k * 128 : (k + 1) * 128],
            compare_op=mybir.AluOpType.is_ge,
            fill=0.0,
            base=0,
            pattern=[[1, 128]],
            channel_multiplier=-1,
        )

    # ---- single-group emitters (fill work injected into attention steps) ----
    def emit_ph1_ob(sp, ob):
        pqk = psum.tile([128, SPAN], F32, name=f"pqk_{ob}_{sp}", tag="ps", bufs=2)
        for c in range(NDM):
            nc.tensor.matmul(
                pqk[:],
                wqkb[:, c, ob * 128 : (ob + 1) * 128],
                xTb[:, c, sp * SPAN : (sp + 1) * SPAN],
                start=(c == 0),
                stop=(c == NDM - 1),
            )
        nc.vector.tensor_scalar_add(
            qkT[ob][:, sp * SPAN : (sp + 1) * SPAN], pqk[:], bqk[ob][:]
        )

    def emit_vproj_sb(sb):
        pv = psum.tile([128, DQ], F32, name=f"pv_{sb}", tag="ps", bufs=2)
        for c in range(NDM):
            nc.tensor.matmul(
                pv[:],
                xTb[:, c, sb * 128 : (sb + 1) * 128],
                wvb[:, c, :],
                start=(c == 0),
                stop=(c == NDM - 1),
            )
        vdst = vp[sb][:, 0 : HPC * VW].rearrange("p (h w) -> p h w", w=VW)[:, :, 0:DH]
        nc.vector.tensor_add(
            vdst,
            pv[:].rearrange("p (h d) -> p h d", d=DH),
            bv[:].rearrange("p (h d) -> p h d", d=DH),
        )

    def emit_outproj(qb, dma_engine=None, split_dma=False):
        ob_t = work.tile([128, DM], BF16, name=f"ob_{qb}", tag="ob", bufs=4)
        eng = dma_engine or nc.sync
        for nh in range(NOUT):
            pot = psum.tile([128, OW], F32, name=f"pot_{qb}_{nh}", tag="ps", bufs=2)
            for c in range(NHD):
                nc.tensor.matmul(
                    pot[:],
                    OT[c][:, qb * 128 : (qb + 1) * 128],
                    wob[:, c, nh * OW : (nh + 1) * OW],
                    start=(c == 0),
                    stop=(c == NHD - 1),
                )
            if nh % 2 == 0:
                nc.scalar.copy(ob_t[:, nh * OW : (nh + 1) * OW], pot[:])
            else:
                nc.vector.tensor_copy(ob_t[:, nh * OW : (nh + 1) * OW], pot[:])
            if split_dma:
                # each half leaves as soon as its PSUM copy lands (tail only:
                # shrinks the post-PE drain to the last 128KB transfer)
                eng.dma_start(
                    io["out"][qb * 128 : (qb + 1) * 128, nh * OW : (nh + 1) * OW],
                    ob_t[:, nh * OW : (nh + 1) * OW],
                )
        if not split_dma:
            eng.dma_start(io["out"][qb * 128 : (qb + 1) * 128, :], ob_t[:])

    # fill schedule: (sp, i) -> list of thunks. Spread so per-step PE work
    # roughly matches the ACT exp pace; late clusters starve the ps ring.
    fills = {}

    def add_fill(sp, i, fn):
        fills.setdefault((sp, i), []).append(fn)

    # coverage: every step of every span carries ~one fill chain so the PE
    # never idles waiting on the ACT exp pace (idle also drops PE pstate)
    for j in range(4):
        add_fill(0, 1 + j, (lambda jj: lambda: emit_vproj_sb(jj))(j))
        add_fill(0, 3 + j, (lambda jj: lambda: emit_ph1_ob(1, jj))(j))
        add_fill(1, 2 * j, (lambda jj: lambda: emit_vproj_sb(4 + jj))(j))
        add_fill(1, 1 + 2 * j, (lambda jj: lambda: emit_ph1_ob(2, jj))(j))
        add_fill(2, 3 + 2 * j, (lambda jj: lambda: emit_ph1_ob(3, jj))(j))
        add_fill(2, 4 + 2 * j, (lambda jj: lambda: emit_vproj_sb(8 + jj))(j))
        add_fill(3, 2 + 2 * j, (lambda jj: lambda: emit_vproj_sb(12 + jj))(j))
    add_fill(1, 8, lambda: emit_outproj(0))
    add_fill(1, 9, lambda: emit_outproj(1))
    add_fill(1, 10, lambda: emit_outproj(2))
    add_fill(2, 2, lambda: emit_outproj(3))
    add_fill(2, 11, lambda: emit_outproj(4))
    add_fill(2, 12, lambda: emit_outproj(5))
    add_fill(2, 13, lambda: emit_outproj(6))
    add_fill(2, 14, lambda: emit_outproj(7))
    add_fill(3, 3, lambda: emit_outproj(8))
    add_fill(3, 10, lambda: emit_outproj(9))
    add_fill(3, 12, lambda: emit_outproj(10))
    add_fill(3, 14, lambda: emit_outproj(11))

    # normalize chain of span sp runs as fill in span sp+1's first steps
    state = {}

    def norm_recip_fill(spp):
        def f():
            den, pos = state[spp]
            state[spp, "denr"] = emit_norm_recip(spp, den)

        return f

    def norm_head_fill(spp, h):
        def f():
            den, pos = state[spp]
            emit_norm_head(spp, h, state[spp, "denr"], pos)

        return f

    for spp in range(NSPAN - 1):
        add_fill(spp + 1, 0, norm_recip_fill(spp))
        add_fill(spp + 1, 0, norm_head_fill(spp, 0))
        add_fill(spp + 1, 0, norm_head_fill(spp, 1))
        add_fill(spp + 1, 1, norm_head_fill(spp, 2))
        add_fill(spp + 1, 1, norm_head_fill(spp, 3))

    def emit_attn(sp):
        nsb = (sp + 1) * SBS
        is_last = sp == NSPAN - 1
        pos = {}
        pts = {}
        den = work.tile(
            [32 * (HPC - 1) + 1, SPAN], F32, name=f"den_{sp}", tag="den", bufs=1
        )

        def emit_scores_pair(i, pair):
            d = i - sp * SBS
            c0 = max(d, 0) * 128  # causal col offset within the span
            if pair == 0:
                pt = work.tile(
                    [128, HPC, SPAN], BF16, name=f"pt_{sp}_{i}", tag="pt", bufs=LAG + 2
                )
                pts[i] = pt
            pt = pts[i]
            ps = psum.tile(
                [128, 2, SPAN], F32, name=f"ps_{sp}_{i}_{pair}", tag="ps", bufs=2
            )
            for sub in range(2):
                h = pair * 2 + sub
                qt = qkT[h // 2]
                kt = qkT[NQK // 2 + h // 2]
                qrow = (h % 2) * 64
                nc.tensor.matmul(
                    ps[:, sub, c0:SPAN],
                    kt[qrow : qrow + 64, i * 128 : (i + 1) * 128],
                    qt[qrow : qrow + 64, sp * SPAN + c0 : (sp + 1) * SPAN],
                    start=True,
                    stop=True,
                )
            nc.scalar.activation(
                pt[:, 2 * pair : 2 * pair + 2, c0:SPAN],
                ps[:, :, c0:SPAN],
                AF.Exp,
                scale=0.125,
            )
            if pair == 1 and d >= 0:
                # mask the triangular diagonal 128-col stripe for all 4 heads
                # (on Pool: SBUF-only operands, keeps DVE free for PSUM work)
                nc.gpsimd.tensor_mul(
                    pt[:, :, c0 : c0 + 128],
                    pt[:, :, c0 : c0 + 128],
                    tri4[:].rearrange("p (h w) -> p h w", w=128),
                )

        def emit_pv_pair(i, pair):
            d = i - sp * SBS
            c0 = max(d, 0) * 128
            pt = pts[i] if pair == 0 else pts.pop(i)
            for sub in range(2):
                h = pair * 2 + sub
                if i == 0:
                    pos[h] = psum.tile(
                        [VW, SPAN], F32, name=f"po_{h}_{sp}", tag="acc", bufs=4
                    )
                nc.tensor.matmul(
                    pos[h][:, c0:SPAN],
                    vp[i][:, h * VW : (h + 1) * VW],
                    pt[:, h, c0:SPAN],
                    start=(i == 0),
                    stop=(i == nsb - 1),
                )
                if i == nsb - 1 and not is_last:
                    # den extraction on ACT: it is in its end-of-span lull
                    nc.scalar.copy(
                        den[32 * h : 32 * h + 1, :], pos[h][VW - 1 : VW, :]
                    )

        def emit_tail_quarter(q):
            # span-3 cols q*128:(q+1)*128 are final after PV k-block 12+q:
            # normalize + out-proj this quarter while later PV blocks run.
            # den hop via ACT (idle here), per-head recip at partition 0, no
            # rtmp: DVE only does recip + the normalize multiplies.
            cs = q * 128
            ce = cs + 128
            dts = []
            for h in range(HPC):
                dt = work.tile([1, 128], F32, name=f"dT_{h}_{q}", tag="dt", bufs=8)
                nc.scalar.copy(dt[:], pos[h][VW - 1 : VW, cs:ce])
                dts.append(dt)
            for h in range(HPC):
                dr = work.tile([1, 128], F32, name=f"drT_{h}_{q}", tag="dr", bufs=8)
                nc.vector.reciprocal_approx_fast(dr[:], dts[h][:])
                recb = work.tile([DH, 128], F32, name=f"rbT_{h}_{q}", tag="recb", bufs=2)
                nc.gpsimd.partition_broadcast(recb[:], dr[0:1, :])
                ot_tile = OT[(h * DH) // 128]
                orow = (h * DH) % 128
                nc.vector.tensor_mul(
                    ot_tile[orow : orow + DH, sp * SPAN + cs : sp * SPAN + ce],
                    pos[h][0:DH, cs:ce],
                    recb[:],
                )
            emit_outproj(
                12 + q,
                dma_engine=nc.scalar if q % 2 else nc.sync,
                split_dma=True,
            )

        for i in range(nsb + LAG):
            if i < nsb:
                emit_scores_pair(i, 0)
            if i >= LAG:
                emit_pv_pair(i - LAG, 0)
            if i < nsb:
                emit_scores_pair(i, 1)
            if i >= LAG:
                emit_pv_pair(i - LAG, 1)
            for fn in fills.get((sp, i), []):
                fn()
            if i >= LAG and is_last and i - LAG >= nsb - 4:
                emit_tail_quarter(i - LAG - (nsb - 4))

        return den, pos

    def emit_norm_recip(sp, den):
        denr = work.tile(
            [32 * (HPC - 1) + 1, SPAN], F32, name=f"denr_{sp}", tag="denr", bufs=1
        )
        nc.vector.reciprocal_approx_fast(denr[:], den[:])
        return denr

    def emit_norm_head(sp, h, denr, pos):
        # normalize straight out of the PV PSUM accumulator (no oraw copy)
        ot_tile = OT[(h * DH) // 128]
        orow = (h * DH) % 128
        rtmp = work.tile([1, SPAN], F32, name=f"rtmp_{h}_{sp}", tag="rtmp", bufs=2)
        nc.vector.tensor_copy(rtmp[:], denr[32 * h : 32 * h + 1, :])
        recb = work.tile([DH, SPAN], F32, name=f"recb_{h}_{sp}", tag="recb", bufs=2)
        nc.gpsimd.partition_broadcast(recb[:], rtmp[0:1, :])
        nc.vector.tensor_mul(
            ot_tile[orow : orow + DH, sp * SPAN : (sp + 1) * SPAN],
            pos[h][0:DH, :],
            recb[:],
        )

    # ---- flat emission: span-0 qk projections, then the attention stream ----
    for ob in range(NQK):
        emit_ph1_ob(0, ob)
    for sp in range(NSPAN):
        state[sp] = emit_attn(sp)


_NC_CACHE = {}


def _get_compiled():
    if "nc" not in _NC_CACHE:
        nc = bacc.Bacc(
            "TRN2", target_bir_lowering=False, debug=False, num_devices=N_CORES
        )
        io = _declare_io(nc)
        with tile.TileContext(nc) as tc, ExitStack() as ctx:
            _build(ctx, tc, io)
        nc.compile()
        _NC_CACHE["nc"] = nc
    return _NC_CACHE["nc"]


def _prep_core_inputs(x, W_qkv, b_qkv, W_out, b_out, core_id):
    g = core_id // CPG
    lane = core_id % CPG
    h0 = lane * HPC
    r = slice(h0 * DH, (h0 + HPC) * DH)
    Wq = W_qkv[0 * DM : 1 * DM, :][r, :]
    Wk = W_qkv[1 * DM : 2 * DM, :][r, :]
    Wv = W_qkv[2 * DM : 3 * DM, :][r, :]
    bq = b_qkv[0 * DM + h0 * DH : 0 * DM + (h0 + HPC) * DH]
    bk = b_qkv[1 * DM + h0 * DH : 1 * DM + (h0 + HPC) * DH]
    bv_ = b_qkv[2 * DM + h0 * DH : 2 * DM + (h0 + HPC) * DH]
    return {
        "xT": np.ascontiguousarray(x[g].T.astype(ml_dtypes.bfloat16)),
        "wqkT": np.ascontiguousarray(
            np.concatenate([Wq.T, Wk.T], axis=1).astype(ml_dtypes.bfloat16)
        ),
        "wvT": np.ascontiguousarray(Wv.T.astype(ml_dtypes.bfloat16)),
        "woT": np.ascontiguousarray(W_out[:, r].T.astype(ml_dtypes.bfloat16)),
        "bqk": np.concatenate([bq, bk]).reshape(2 * DQ, 1).astype(np.float32),
        "bv": np.ascontiguousarray(bv_.reshape(1, DQ).astype(np.float32)),
    }


def kernel(x, W_qkv, b_qkv, W_out, b_out, _trace=False):
    x = np.asarray(x)
    W_qkv = np.asarray(W_qkv)
    b_qkv = np.asarray(b_qkv)
    W_out = np.asarray(W_out)
    b_out = np.asarray(b_out)

    nc = _get_compiled()
    in_maps = [
        _prep_core_inputs(x, W_qkv, b_qkv, W_out, b_out, c) for c in range(N_CORES)
    ]
    res = run_bass_kernel_spmd(nc, in_maps, list(range(N_CORES)), trace=_trace)

    out = np.empty((B, S, DM), dtype=np.float32)
    for g in range(B):
        acc = res.results[g * CPG]["out"].astype(np.float32)
        for lane in range(1, CPG):
            acc = acc + res.results[g * CPG + lane]["out"]
        out[g] = acc + b_out[None, :].astype(np.float32)

    if _trace:
        kernel.last_exec_time_ns = res.exec_time_ns
        kernel.last_results = res
    return out


# revision 16
# speedup vs baseline: 1.3681x; 1.3681x over previous
"""Multi-head causal self-attention (torch nn.MultiheadAttention semantics)
on 8 Trainium2 NeuronCores.

Problem: x [2, 2048, 1024], 16 heads, head dim 64, fp32, causal, p_drop=0.

Sharding: 2 batch groups x 4-way head tensor-parallel.
  core c: batch b = c // 4, heads [lane*4, lane*4+4) with lane = c % 4.
Each core computes q/k/v projections for its 4 heads, flash-style causal
attention (S^T score layout, no-max softmax — scores are O(1) here), and its
partial out-projection. The host sums the 4 partials per batch and adds b_out.

v5 (trace-driven, from v3 @ 202us, v4b @ 188.5us):
  - Only sync (SP) and scalar (ACT) queues have fast hardware DMA (HWDGE,
    ~250GB/s multi-ring); gpsimd dma_start is a slow single-ring software
    path (~50GB/s). Critical prefix (wqk[0:2], x[0:2] of span 0, wv) issues
    on the scalar queue (ACT idle until first exp ~14us), the bulk on sync,
    only the tiny biases on gpsimd.
  - reciprocal -> reciprocal_approx_fast (~5x): the 3.35us iterative divide
    head-of-line blocked the PSUM-freeing bias-adds at span boundaries.
  - No oraw copies: normalize muls (DVE) read the PV PSUM accumulators
    directly (GPSIMD cannot touch PSUM; ACT copies were on the exp pacer).
  - W_out/OT bf16 (same PE rate at 512 free dim), b_v broadcast on chip.
  - LAG=3: PV trails scores more so PE rarely waits on the exp stream.
  - Fill chains spread evenly across steps (ACT exp pace is ~2.1us/step;
    clustered fills let the ps PSUM ring drain and stall later scores).
  - Tail: span-3 PV columns finalize progressively (cols q*128:(q+1)*128
    are final after k-block 12+q), so each 128-col quarter's den/normalize/
    out-proj/DMA chain is emitted right after that PV step, overlapping the
    remaining PV work; last out DMAs ride the fast scalar/sync queues.
"""

import os
from contextlib import ExitStack

import ml_dtypes
import numpy as np

import concourse.bass as bass
import concourse.tile as tile
from concourse import bacc, mybir
from concourse.bass_utils import run_bass_kernel_spmd

F32 = mybir.dt.float32
BF16 = mybir.dt.bfloat16
AF = mybir.ActivationFunctionType

B = 2
S = 2048
DM = 1024
N_HEADS = 16
DH = 64
N_CORES = 8
CPG = 4  # cores per group (tensor-parallel width over heads)
HPC = N_HEADS // CPG  # heads per core
DQ = HPC * DH
SPAN = 512
SB = 128
NDM = DM // 128
NSPAN = S // SPAN
NSB = S // SB
SBS = SPAN // SB
NQK = 2 * DQ // 128
NHD = DQ // 128
VW = DH + 1
OW = 512
NOUT = DM // OW
LAG = 3  # PV trails scores by this many sk blocks


def _declare_io(nc):
    t = {}
    t["xT"] = nc.dram_tensor("xT", [DM, S], BF16, kind="ExternalInput").ap()
    t["wqkT"] = nc.dram_tensor("wqkT", [DM, 2 * DQ], BF16, kind="ExternalInput").ap()
    t["wvT"] = nc.dram_tensor("wvT", [DM, DQ], BF16, kind="ExternalInput").ap()
    t["woT"] = nc.dram_tensor("woT", [DQ, DM], BF16, kind="ExternalInput").ap()
    t["bqk"] = nc.dram_tensor("bqk", [2 * DQ, 1], F32, kind="ExternalInput").ap()
    t["bv"] = nc.dram_tensor("bv", [1, DQ], F32, kind="ExternalInput").ap()
    t["out"] = nc.dram_tensor("out", [S, DM], BF16, kind="ExternalOutput").ap()
    return t


def _build(ctx: ExitStack, tc: tile.TileContext, io: dict):
    nc = tc.nc

    const = ctx.enter_context(tc.tile_pool(name="const", bufs=1))
    work = ctx.enter_context(tc.tile_pool(name="work", bufs=1))
    psum = ctx.enter_context(tc.tile_pool(name="psum", bufs=1, space="PSUM"))

    # ---- persistent tiles ----
    xTb = const.tile([128, NDM, S], BF16, name="xTb")
    wqkb = const.tile([128, NDM, 2 * DQ], BF16, name="wqkb")
    wvb = const.tile([128, NDM, DQ], BF16, name="wvb")
    wob = const.tile([128, NHD, DM], BF16, name="wob")
    bqkb = const.tile([128, NQK], F32, name="bqkb")
    bqk = [bqkb[:, b : b + 1] for b in range(NQK)]
    bvs = const.tile([1, DQ], F32, name="bvs")
    bv = const.tile([128, DQ], F32, name="bv")
    qkT = [const.tile([128, S], BF16, name=f"qkT{b}") for b in range(NQK)]
    vp = [const.tile([128, HPC * VW], BF16, name=f"vp{sb}") for sb in range(NSB)]
    OT = [const.tile([128, S], BF16, name=f"OT{c}") for c in range(NHD)]

    # ---- input DMAs (fast HWDGE queues: sync + scalar; gpsimd is slow).
    # Two balanced streams: first-needed piece leads each queue so the
    # ph1(0) chains can chase c-chunk arrivals. ----
    xTd = io["xT"].rearrange("(c p) s -> p c s", p=128)
    wqkd = io["wqkT"].rearrange("(c p) w -> p c w", p=128)
    nc.scalar.dma_start(wqkb[:, 0:2, :], wqkd[:, 0:2, :])
    nc.scalar.dma_start(wqkb[:, 2:NDM, :], wqkd[:, 2:NDM, :])
    nc.scalar.dma_start(wvb[:], io["wvT"].rearrange("(c p) w -> p c w", p=128))
    nc.sync.dma_start(xTb[:, 0:2, 0:SPAN], xTd[:, 0:2, 0:SPAN])
    nc.sync.dma_start(xTb[:, 2:NDM, 0:SPAN], xTd[:, 2:NDM, 0:SPAN])
    for sp in range(1, NSPAN):
        nc.sync.dma_start(
            xTb[:, :, sp * SPAN : (sp + 1) * SPAN],
            xTd[:, :, sp * SPAN : (sp + 1) * SPAN],
        )
    nc.sync.dma_start(wob[:], io["woT"].rearrange("(c p) w -> p c w", p=128))
    nc.gpsimd.dma_start(bqkb[:], io["bqk"].rearrange("(b p) o -> p (b o)", p=128))
    nc.gpsimd.dma_start(bvs[:], io["bv"][:])

    # ---- constants: bv broadcast, vp ones-columns, replicated causal tri ----
    nc.gpsimd.partition_broadcast(bv[:], bvs[0:1, :])
    for sb in range(NSB):
        nc.gpsimd.memset(vp[sb][:, DH : HPC * VW : VW], 1.0)
    # tri4 = 4 side-by-side copies of tri[r, c] = (c - r >= 0)
    tri4 = const.tile([128, 4 * 128], BF16, name="tri4")
    nc.gpsimd.memset(tri4[:], 1.0)
    for k in range(4):
        nc.gpsimd.affine_select(
            out=tri4[:, k * 128 : (k + 1) * 128],
            in_=tri4[:, k * 128 : (k + 1) * 128],
            compare_op=mybir.AluOpType.is_ge,
            fill=0.0,
            base=0,
            pattern=[[1, 128]],
            channel_multiplier=-1,
        )

    # ---- single-group emitters (fill work injected into attention steps) ----
    def emit_ph1_ob(sp, ob):
        pqk = psum.tile([128, SPAN], F32, name=f"pqk_{ob}_{sp}", tag="ps", bufs=2)
        for c in range(NDM):
            nc.tensor.matmul(
                pqk[:],
                wqkb[:, c, ob * 128 : (ob + 1) * 128],
                xTb[:, c, sp * SPAN : (sp + 1) * SPAN],
                start=(c == 0),
                stop=(c == NDM - 1),
            )
        nc.vector.tensor_scalar_add(
            qkT[ob][:, sp * SPAN : (sp + 1) * SPAN], pqk[:], bqk[ob][:]
        )

    def emit_vproj_sb(sb):
        pv = psum.tile([128, DQ], F32, name=f"pv_{sb}", tag="ps", bufs=2)
        for c in range(NDM):
            nc.tensor.matmul(
                pv[:],
                xTb[:, c, sb * 128 : (sb + 1) * 128],
                wvb[:, c, :],
                start=(c == 0),
                stop=(c == NDM - 1),
            )
        vdst = vp[sb][:, 0 : HPC * VW].rearrange("p (h w) -> p h w", w=VW)[:, :, 0:DH]
        nc.vector.tensor_add(
            vdst,
            pv[:].rearrange("p (h d) -> p h d", d=DH),
            bv[:].rearrange("p (h d) -> p h d", d=DH),
        )

    def emit_outproj(qb, dma_engine=None, split_dma=False):
        ob_t = work.tile([128, DM], BF16, name=f"ob_{qb}", tag="ob", bufs=4)
        eng = dma_engine or nc.sync
        for nh in range(NOUT):
            pot = psum.tile([128, OW], F32, name=f"pot_{qb}_{nh}", tag="ps", bufs=2)
            for c in range(NHD):
                nc.tensor.matmul(
                    pot[:],
                    OT[c][:, qb * 128 : (qb + 1) * 128],
                    wob[:, c, nh * OW : (nh + 1) * OW],
                    start=(c == 0),
                    stop=(c == NHD - 1),
                )
            if nh % 2 == 0:
                nc.scalar.copy(ob_t[:, nh * OW : (nh + 1) * OW], pot[:])
            else:
                nc.vector.tensor_copy(ob_t[:, nh * OW : (nh + 1) * OW], pot[:])
            if split_dma:
                # each half leaves as soon as its PSUM copy lands (tail only:
                # shrinks the post-PE drain to the last 128KB transfer)
                eng.dma_start(
                    io["out"][qb * 128 : (qb + 1) * 128, nh * OW : (nh + 1) * OW],
                    ob_t[:, nh * OW : (nh + 1) * OW],
                )
        if not split_dma:
            eng.dma_start(io["out"][qb * 128 : (qb + 1) * 128, :], ob_t[:])

    # fill schedule: (sp, i) -> list of thunks. Spread so per-step PE work
    # roughly matches the ACT exp pace; late clusters starve the ps ring.
    fills = {}

    def add_fill(sp, i, fn):
        fills.setdefault((sp, i), []).append(fn)

    # coverage: every step of every span carries ~one fill chain so the PE
    # never idles waiting on the ACT exp pace (idle also drops PE pstate)
    for j in range(4):
        add_fill(0, 1 + j, (lambda jj: lambda: emit_vproj_sb(jj))(j))
        add_fill(0, 3 + j, (lambda jj: lambda: emit_ph1_ob(1, jj))(j))
        add_fill(1, 2 * j, (lambda jj: lambda: emit_vproj_sb(4 + jj))(j))
        add_fill(1, 1 + 2 * j, (lambda jj: lambda: emit_ph1_ob(2, jj))(j))
        add_fill(2, 3 + 2 * j, (lambda jj: lambda: emit_ph1_ob(3, jj))(j))
        add_fill(2, 4 + 2 * j, (lambda jj: lambda: emit_vproj_sb(8 + jj))(j))
        add_fill(3, 2 + 2 * j, (lambda jj: lambda: emit_vproj_sb(12 + jj))(j))
    add_fill(1, 8, lambda: emit_outproj(0))
    add_fill(1, 9, lambda: emit_outproj(1))
    add_fill(1, 10, lambda: emit_outproj(2))
    add_fill(2, 2, lambda: emit_outproj(3))
    add_fill(2, 11, lambda: emit_outproj(4))
    add_fill(2, 12, lambda: emit_outproj(5))
    add_fill(2, 13, lambda: emit_outproj(6))
    add_fill(2, 14, lambda: emit_outproj(7))
    add_fill(3, 3, lambda: emit_outproj(8))
    add_fill(3, 10, lambda: emit_outproj(9))
    add_fill(3, 12, lambda: emit_outproj(10))
    add_fill(3, 14, lambda: emit_outproj(11))

    # normalize chain of span sp runs as fill in span sp+1's first steps
    state = {}

    def norm_recip_fill(spp):
        def f():
            den, pos = state[spp]
            state[spp, "denr"] = emit_norm_recip(spp, den)

        return f

    def norm_head_fill(spp, h):
        def f():
            den, pos = state[spp]
            emit_norm_head(spp, h, state[spp, "denr"], pos)

        return f

    for spp in range(NSPAN - 1):
        add_fill(spp + 1, 0, norm_recip_fill(spp))
        add_fill(spp + 1, 0, norm_head_fill(spp, 0))
        add_fill(spp + 1, 0, norm_head_fill(spp, 1))
        add_fill(spp + 1, 1, norm_head_fill(spp, 2))
        add_fill(spp + 1, 1, norm_head_fill(spp, 3))

    def emit_attn(sp):
        nsb = (sp + 1) * SBS
        is_last = sp == NSPAN - 1
        pos = {}
        pts = {}
        den = work.tile(
            [32 * (HPC - 1) + 1, SPAN], F32, name=f"den_{sp}", tag="den", bufs=1
        )

        def emit_scores_pair(i, pair):
            d = i - sp * SBS
            c0 = max(d, 0) * 128  # causal col offset within the span
            if pair == 0:
                pt = work.tile(
                    [128, HPC, SPAN], BF16, name=f"pt_{sp}_{i}", tag="pt", bufs=LAG + 2
                )
                pts[i] = pt
            pt = pts[i]
            ps = psum.tile(
                [128, 2, SPAN], F32, name=f"ps_{sp}_{i}_{pair}", tag="ps", bufs=2
            )
            for sub in range(2):
                h = pair * 2 + sub
                qt = qkT[h // 2]
                kt = qkT[NQK // 2 + h // 2]
                qrow = (h % 2) * 64
                nc.tensor.matmul(
                    ps[:, sub, c0:SPAN],
                    kt[qrow : qrow + 64, i * 128 : (i + 1) * 128],
                    qt[qrow : qrow + 64, sp * SPAN + c0 : (sp + 1) * SPAN],
                    start=True,
                    stop=True,
                )
            nc.scalar.activation(
                pt[:, 2 * pair : 2 * pair + 2, c0:SPAN],
                ps[:, :, c0:SPAN],
                AF.Exp,
                scale=0.125,
            )
            if pair == 1 and d >= 0:
                # mask the triangular diagonal 128-col stripe for all 4 heads
                # (DVE; Pool tensor ops cost ~1.2us + library reloads)
                nc.vector.tensor_mul(
                    pt[:, :, c0 : c0 + 128],
                    pt[:, :, c0 : c0 + 128],
                    tri4[:].rearrange("p (h w) -> p h w", w=128),
                )

        def emit_pv_pair(i, pair):
            d = i - sp * SBS
            c0 = max(d, 0) * 128
            pt = pts[i] if pair == 0 else pts.pop(i)
            for sub in range(2):
                h = pair * 2 + sub
                if i == 0:
                    pos[h] = psum.tile(
                        [VW, SPAN], F32, name=f"po_{h}_{sp}", tag="acc", bufs=4
                    )
                nc.tensor.matmul(
                    pos[h][:, c0:SPAN],
                    vp[i][:, h * VW : (h + 1) * VW],
                    pt[:, h, c0:SPAN],
                    start=(i == 0),
                    stop=(i == nsb - 1),
                )
                if i == nsb - 1 and not is_last:
                    # den extraction on ACT: it is in its end-of-span lull
                    nc.scalar.copy(
                        den[32 * h : 32 * h + 1, :], pos[h][VW - 1 : VW, :]
                    )

        def emit_tail_quarter(q):
            # span-3 cols q*128:(q+1)*128 are final after PV k-block 12+q:
            # normalize + out-proj this quarter while later PV blocks run.
            # den hop via ACT (idle here), per-head recip at partition 0, no
            # rtmp: DVE only does recip + the normalize multiplies.
            cs = q * 128
            ce = cs + 128
            dts = []
            for h in range(HPC):
                dt = work.tile([1, 128], F32, name=f"dT_{h}_{q}", tag="dt", bufs=8)
                nc.scalar.copy(dt[:], pos[h][VW - 1 : VW, cs:ce])
                dts.append(dt)
            for h in range(HPC):
                dr = work.tile([1, 128], F32, name=f"drT_{h}_{q}", tag="dr", bufs=8)
                nc.vector.reciprocal_approx_fast(dr[:], dts[h][:])
                recb = work.tile([DH, 128], F32, name=f"rbT_{h}_{q}", tag="recb", bufs=2)
                nc.gpsimd.partition_broadcast(recb[:], dr[0:1, :])
                ot_tile = OT[(h * DH) // 128]
                orow = (h * DH) % 128
                nc.vector.tensor_mul(
                    ot_tile[orow : orow + DH, sp * SPAN + cs : sp * SPAN + ce],
                    pos[h][0:DH, cs:ce],
                    recb[:],
                )
            emit_outproj(
                12 + q,
                dma_engine=nc.scalar if q % 2 else nc.sync,
                split_dma=True,
            )

        for i in range(nsb + LAG):
            if i < nsb:
                emit_scores_pair(i, 0)
            if i >= LAG:
                emit_pv_pair(i - LAG, 0)
            if i < nsb:
                emit_scores_pair(i, 1)
            if i >= LAG:
                emit_pv_pair(i - LAG, 1)
            for fn in fills.get((sp, i), []):
                fn()
            if i >= LAG and is_last and i - LAG >= nsb - 4:
                emit_tail_quarter(i - LAG - (nsb - 4))

        return den, pos

    def emit_norm_recip(sp, den):
        denr = work.tile(
            [32 * (HPC - 1) + 1, SPAN], F32, name=f"denr_{sp}", tag="denr", bufs=1
        )
        nc.vector.reciprocal_approx_fast(denr[:], den[:])
        return denr

    def emit_norm_head(sp, h, denr, pos):
        # normalize straight out of the PV PSUM accumulator (no oraw copy)
        ot_tile = OT[(h * DH) // 128]
        orow = (h * DH) % 128
        rtmp = work.tile([1, SPAN], F32, name=f"rtmp_{h}_{sp}", tag="rtmp", bufs=2)
        nc.vector.tensor_copy(rtmp[:], denr[32 * h : 32 * h + 1, :])
        recb = work.tile([DH, SPAN], F32, name=f"recb_{h}_{sp}", tag="recb", bufs=2)
        nc.gpsimd.partition_broadcast(recb[:], rtmp[0:1, :])
        nc.vector.tensor_mul(
            ot_tile[orow : orow + DH, sp * SPAN : (sp + 1) * SPAN],
            pos[h][0:DH, :],
            recb[:],
        )

    # ---- flat emission: span-0 qk projections, then the attention stream ----
    for ob in range(NQK):
        emit_ph1_ob(0, ob)
    for sp in range(NSPAN):
        state[sp] = emit_attn(sp)


_NC_CACHE = {}


def _get_compiled():
    if "nc" not in _NC_CACHE:
        nc = bacc.Bacc(
            "TRN2", target_bir_lowering=False, debug=False, num_devices=N_CORES
        )
        io = _declare_io(nc)
        with tile.TileContext(nc) as tc, ExitStack() as ctx:
            _build(ctx, tc, io)
        nc.compile()
        _NC_CACHE["nc"] = nc
    return _NC_CACHE["nc"]


def _prep_core_inputs(x, W_qkv, b_qkv, W_out, b_out, core_id):
    g = core_id // CPG
    lane = core_id % CPG
    h0 = lane * HPC
    r = slice(h0 * DH, (h0 + HPC) * DH)
    Wq = W_qkv[0 * DM : 1 * DM, :][r, :]
    Wk = W_qkv[1 * DM : 2 * DM, :][r, :]
    Wv = W_qkv[2 * DM : 3 * DM, :][r, :]
    bq = b_qkv[0 * DM + h0 * DH : 0 * DM + (h0 + HPC) * DH]
    bk = b_qkv[1 * DM + h0 * DH : 1 * DM + (h0 + HPC) * DH]
    bv_ = b_qkv[2 * DM + h0 * DH : 2 * DM + (h0 + HPC) * DH]
    return {
        "xT": np.ascontiguousarray(x[g].T.astype(ml_dtypes.bfloat16)),
        "wqkT": np.ascontiguousarray(
            np.concatenate([Wq.T, Wk.T], axis=1).astype(ml_dtypes.bfloat16)
        ),
        "wvT": np.ascontiguousarray(Wv.T.astype(ml_dtypes.bfloat16)),
        "woT": np.ascontiguousarray(W_out[:, r].T.astype(ml_dtypes.bfloat16)),
        "bqk": np.concatenate([bq, bk]).reshape(2 * DQ, 1).astype(np.float32),
        "bv": np.ascontiguousarray(bv_.reshape(1, DQ).astype(np.float32)),
    }


def kernel(x, W_qkv, b_qkv, W_out, b_out, _trace=False):
    x = np.asarray(x)
    W_qkv = np.asarray(W_qkv)
    b_qkv = np.asarray(b_qkv)
    W_out = np.asarray(W_out)
    b_out = np.asarray(b_out)

    nc = _get_compiled()
    in_maps = [
        _prep_core_inputs(x, W_qkv, b_qkv, W_out, b_out, c) for c in range(N_CORES)
    ]
    res = run_bass_kernel_spmd(nc, in_maps, list(range(N_CORES)), trace=_trace)

    out = np.empty((B, S, DM), dtype=np.float32)
    for g in range(B):
        acc = res.results[g * CPG]["out"].astype(np.float32)
        for lane in range(1, CPG):
            acc = acc + res.results[g * CPG + lane]["out"]
        out[g] = acc + b_out[None, :].astype(np.float32)

    if _trace:
        kernel.last_exec_time_ns = res.exec_time_ns
        kernel.last_results = res
    return out


# revision 19
# speedup vs baseline: 1.3804x; 1.0090x over previous
"""Multi-head causal self-attention (torch nn.MultiheadAttention semantics)
on 8 Trainium2 NeuronCores.

Problem: x [2, 2048, 1024], 16 heads, head dim 64, fp32, causal, p_drop=0.

Sharding: 2 batch groups x 4-way head tensor-parallel.
  core c: batch b = c // 4, heads [lane*4, lane*4+4) with lane = c % 4.
Each core computes q/k/v projections for its 4 heads, flash-style causal
attention (S^T score layout, no-max softmax — scores are O(1) here), and its
partial out-projection. The host sums the 4 partials per batch and adds b_out.

v5 (trace-driven, from v3 @ 202us, v4b @ 188.5us):
  - Only sync (SP) and scalar (ACT) queues have fast hardware DMA (HWDGE,
    ~250GB/s multi-ring); gpsimd dma_start is a slow single-ring software
    path (~50GB/s). Critical prefix (wqk[0:2], x[0:2] of span 0, wv) issues
    on the scalar queue (ACT idle until first exp ~14us), the bulk on sync,
    only the tiny biases on gpsimd.
  - reciprocal -> reciprocal_approx_fast (~5x): the 3.35us iterative divide
    head-of-line blocked the PSUM-freeing bias-adds at span boundaries.
  - No oraw copies: normalize muls (DVE) read the PV PSUM accumulators
    directly (GPSIMD cannot touch PSUM; ACT copies were on the exp pacer).
  - W_out/OT bf16 (same PE rate at 512 free dim), b_v broadcast on chip.
  - LAG=3: PV trails scores more so PE rarely waits on the exp stream.
  - Fill chains spread evenly across steps (ACT exp pace is ~2.1us/step;
    clustered fills let the ps PSUM ring drain and stall later scores).
  - Tail: span-3 PV columns finalize progressively (cols q*128:(q+1)*128
    are final after k-block 12+q), so each 128-col quarter's den/normalize/
    out-proj/DMA chain is emitted right after that PV step, overlapping the
    remaining PV work; last out DMAs ride the fast scalar/sync queues.
"""

import os
from contextlib import ExitStack

import ml_dtypes
import numpy as np

import concourse.bass as bass
import concourse.tile as tile
from concourse import bacc, mybir
from concourse.bass_utils import run_bass_kernel_spmd

F32 = mybir.dt.float32
BF16 = mybir.dt.bfloat16
AF = mybir.ActivationFunctionType

B = 2
S = 2048
DM = 1024
N_HEADS = 16
DH = 64
N_CORES = 8
CPG = 4  # cores per group (tensor-parallel width over heads)
HPC = N_HEADS // CPG  # heads per core
DQ = HPC * DH
SPAN = 512
SB = 128
NDM = DM // 128
NSPAN = S // SPAN
NSB = S // SB
SBS = SPAN // SB
NQK = 2 * DQ // 128
NHD = DQ // 128
VW = DH + 1
OW = 512
NOUT = DM // OW
LAG = 3  # PV trails scores by this many sk blocks


def _declare_io(nc):
    t = {}
    t["xT"] = nc.dram_tensor("xT", [DM, S], BF16, kind="ExternalInput").ap()
    t["wqkT"] = nc.dram_tensor("wqkT", [DM, 2 * DQ], BF16, kind="ExternalInput").ap()
    t["wvT"] = nc.dram_tensor("wvT", [DM, DQ], BF16, kind="ExternalInput").ap()
    t["woT"] = nc.dram_tensor("woT", [DQ, DM], BF16, kind="ExternalInput").ap()
    t["bqk"] = nc.dram_tensor("bqk", [2 * DQ, 1], F32, kind="ExternalInput").ap()
    t["bv"] = nc.dram_tensor("bv", [1, DQ], F32, kind="ExternalInput").ap()
    t["out"] = nc.dram_tensor("out", [S, DM], BF16, kind="ExternalOutput").ap()
    return t


def _build(ctx: ExitStack, tc: tile.TileContext, io: dict):
    nc = tc.nc

    const = ctx.enter_context(tc.tile_pool(name="const", bufs=1))
    work = ctx.enter_context(tc.tile_pool(name="work", bufs=1))
    psum = ctx.enter_context(tc.tile_pool(name="psum", bufs=1, space="PSUM"))

    # ---- persistent tiles ----
    xTb = const.tile([128, NDM, S], BF16, name="xTb")
    wqkb = const.tile([128, NDM, 2 * DQ], BF16, name="wqkb")
    wvb = const.tile([128, NDM, DQ], BF16, name="wvb")
    wob = const.tile([128, NHD, DM], BF16, name="wob")
    bqkb = const.tile([128, NQK], F32, name="bqkb")
    bqk = [bqkb[:, b : b + 1] for b in range(NQK)]
    bvs = const.tile([1, DQ], F32, name="bvs")
    bv = const.tile([128, DQ], F32, name="bv")
    qkT = [const.tile([128, S], BF16, name=f"qkT{b}") for b in range(NQK)]
    vp = [const.tile([128, HPC * VW], BF16, name=f"vp{sb}") for sb in range(NSB)]
    OT = [const.tile([128, S], BF16, name=f"OT{c}") for c in range(NHD)]

    # ---- input DMAs (fast HWDGE queues: sync + scalar; gpsimd is slow).
    # Two balanced streams: first-needed piece leads each queue so the
    # ph1(0) chains can chase c-chunk arrivals. ----
    xTd = io["xT"].rearrange("(c p) s -> p c s", p=128)
    wqkd = io["wqkT"].rearrange("(c p) w -> p c w", p=128)
    for c in range(0, NDM, 2):
        nc.scalar.dma_start(wqkb[:, c : c + 2, :], wqkd[:, c : c + 2, :])
    nc.scalar.dma_start(wvb[:], io["wvT"].rearrange("(c p) w -> p c w", p=128))
    for c in range(0, NDM, 2):
        nc.sync.dma_start(xTb[:, c : c + 2, 0:SPAN], xTd[:, c : c + 2, 0:SPAN])
    for sp in range(1, NSPAN):
        nc.sync.dma_start(
            xTb[:, :, sp * SPAN : (sp + 1) * SPAN],
            xTd[:, :, sp * SPAN : (sp + 1) * SPAN],
        )
    nc.sync.dma_start(wob[:], io["woT"].rearrange("(c p) w -> p c w", p=128))
    nc.gpsimd.dma_start(bqkb[:], io["bqk"].rearrange("(b p) o -> p (b o)", p=128))
    nc.gpsimd.dma_start(bvs[:], io["bv"][:])

    # ---- constants: bv broadcast, vp ones-columns, replicated causal tri ----
    nc.gpsimd.partition_broadcast(bv[:], bvs[0:1, :])
    for sb in range(NSB):
        nc.gpsimd.memset(vp[sb][:, DH : HPC * VW : VW], 1.0)
    # tri4 = 4 side-by-side copies of tri[r, c] = (c - r >= 0)
    tri4 = const.tile([128, 4 * 128], BF16, name="tri4")
    nc.gpsimd.memset(tri4[:], 1.0)
    for k in range(4):
        nc.gpsimd.affine_select(
            out=tri4[:, k * 128 : (k + 1) * 128],
            in_=tri4[:, k * 128 : (k + 1) * 128],
            compare_op=mybir.AluOpType.is_ge,
            fill=0.0,
            base=0,
            pattern=[[1, 128]],
            channel_multiplier=-1,
        )

    # ---- PE warmup: dependency-free matmuls on a never-written tile run
    # during the input-DMA wait, ramping the PE pstate (0.65 -> 2.4 GHz
    # after ~3us continuous execution) so ph1(0) runs at full clock ----
    warm_src = const.tile([128, 640], BF16, name="warm_src")
    nc.vector.memset(warm_src[:], 0.0)
    for w in range(8):
        wp = psum.tile([128, SPAN], F32, name=f"warm{w}", tag="ps", bufs=2)
        nc.tensor.matmul(
            wp[:], warm_src[:, 0:128], warm_src[:, 128:640], start=True, stop=True
        )

    # ---- single-group emitters (fill work injected into attention steps) ----
    def emit_ph1_ob(sp, ob):
        pqk = psum.tile([128, SPAN], F32, name=f"pqk_{ob}_{sp}", tag="ps", bufs=2)
        for c in range(NDM):
            nc.tensor.matmul(
                pqk[:],
                wqkb[:, c, ob * 128 : (ob + 1) * 128],
                xTb[:, c, sp * SPAN : (sp + 1) * SPAN],
                start=(c == 0),
                stop=(c == NDM - 1),
            )
        nc.vector.tensor_scalar_add(
            qkT[ob][:, sp * SPAN : (sp + 1) * SPAN], pqk[:], bqk[ob][:]
        )

    def emit_vproj_sb(sb):
        pv = psum.tile([128, DQ], F32, name=f"pv_{sb}", tag="ps", bufs=2)
        for c in range(NDM):
            nc.tensor.matmul(
                pv[:],
                xTb[:, c, sb * 128 : (sb + 1) * 128],
                wvb[:, c, :],
                start=(c == 0),
                stop=(c == NDM - 1),
            )
        vdst = vp[sb][:, 0 : HPC * VW].rearrange("p (h w) -> p h w", w=VW)[:, :, 0:DH]
        nc.vector.tensor_add(
            vdst,
            pv[:].rearrange("p (h d) -> p h d", d=DH),
            bv[:].rearrange("p (h d) -> p h d", d=DH),
        )

    def emit_outproj(qb, dma_engine=None, split_dma=False):
        ob_t = work.tile([128, DM], BF16, name=f"ob_{qb}", tag="ob", bufs=4)
        eng = dma_engine or nc.sync
        for nh in range(NOUT):
            pot = psum.tile([128, OW], F32, name=f"pot_{qb}_{nh}", tag="ps", bufs=2)
            for c in range(NHD):
                nc.tensor.matmul(
                    pot[:],
                    OT[c][:, qb * 128 : (qb + 1) * 128],
                    wob[:, c, nh * OW : (nh + 1) * OW],
                    start=(c == 0),
                    stop=(c == NHD - 1),
                )
            if nh % 2 == 0:
                nc.scalar.copy(ob_t[:, nh * OW : (nh + 1) * OW], pot[:])
            else:
                nc.vector.tensor_copy(ob_t[:, nh * OW : (nh + 1) * OW], pot[:])
            if split_dma:
                # each half leaves as soon as its PSUM copy lands (tail only:
                # shrinks the post-PE drain to the last 128KB transfer)
                eng.dma_start(
                    io["out"][qb * 128 : (qb + 1) * 128, nh * OW : (nh + 1) * OW],
                    ob_t[:, nh * OW : (nh + 1) * OW],
                )
        if not split_dma:
            eng.dma_start(io["out"][qb * 128 : (qb + 1) * 128, :], ob_t[:])

    # fill schedule: (sp, i) -> list of thunks. Spread so per-step PE work
    # roughly matches the ACT exp pace; late clusters starve the ps ring.
    fills = {}

    def add_fill(sp, i, fn):
        fills.setdefault((sp, i), []).append(fn)

    # coverage: every step of every span carries ~one fill chain so the PE
    # never idles waiting on the ACT exp pace (idle also drops PE pstate)
    for j in range(4):
        add_fill(0, 1 + j, (lambda jj: lambda: emit_vproj_sb(jj))(j))
        add_fill(0, 3 + j, (lambda jj: lambda: emit_ph1_ob(1, jj))(j))
        add_fill(1, 2 * j, (lambda jj: lambda: emit_vproj_sb(4 + jj))(j))
        add_fill(1, 1 + 2 * j, (lambda jj: lambda: emit_ph1_ob(2, jj))(j))
        add_fill(2, 3 + 2 * j, (lambda jj: lambda: emit_ph1_ob(3, jj))(j))
        add_fill(2, 4 + 2 * j, (lambda jj: lambda: emit_vproj_sb(8 + jj))(j))
        add_fill(3, 2 + 2 * j, (lambda jj: lambda: emit_vproj_sb(12 + jj))(j))
    add_fill(1, 8, lambda: emit_outproj(0))
    add_fill(1, 9, lambda: emit_outproj(1))
    add_fill(1, 10, lambda: emit_outproj(2))
    add_fill(2, 2, lambda: emit_outproj(3))
    add_fill(2, 11, lambda: emit_outproj(4))
    add_fill(2, 12, lambda: emit_outproj(5))
    add_fill(2, 13, lambda: emit_outproj(6))
    add_fill(2, 14, lambda: emit_outproj(7))
    add_fill(3, 3, lambda: emit_outproj(8))
    add_fill(3, 10, lambda: emit_outproj(9))
    add_fill(3, 12, lambda: emit_outproj(10))
    add_fill(3, 14, lambda: emit_outproj(11))

    # normalize chain of span sp runs as fill in span sp+1's first steps
    state = {}

    def norm_recip_fill(spp):
        def f():
            den, pos = state[spp]
            state[spp, "denr"] = emit_norm_recip(spp, den)

        return f

    def norm_head_fill(spp, h):
        def f():
            den, pos = state[spp]
            emit_norm_head(spp, h, state[spp, "denr"], pos)

        return f

    for spp in range(NSPAN - 1):
        add_fill(spp + 1, 0, norm_recip_fill(spp))
        add_fill(spp + 1, 0, norm_head_fill(spp, 0))
        add_fill(spp + 1, 0, norm_head_fill(spp, 1))
        add_fill(spp + 1, 1, norm_head_fill(spp, 2))
        add_fill(spp + 1, 1, norm_head_fill(spp, 3))

    def emit_attn(sp):
        nsb = (sp + 1) * SBS
        is_last = sp == NSPAN - 1
        pos = {}
        pts = {}
        den = work.tile(
            [32 * (HPC - 1) + 1, SPAN], F32, name=f"den_{sp}", tag="den", bufs=1
        )

        def emit_scores_pair(i, pair):
            d = i - sp * SBS
            c0 = max(d, 0) * 128  # causal col offset within the span
            if pair == 0:
                pt = work.tile(
                    [128, HPC, SPAN], BF16, name=f"pt_{sp}_{i}", tag="pt", bufs=LAG + 2
                )
                pts[i] = pt
            pt = pts[i]
            ps = psum.tile(
                [128, 2, SPAN], F32, name=f"ps_{sp}_{i}_{pair}", tag="ps", bufs=2
            )
            for sub in range(2):
                h = pair * 2 + sub
                qt = qkT[h // 2]
                kt = qkT[NQK // 2 + h // 2]
                qrow = (h % 2) * 64
                nc.tensor.matmul(
                    ps[:, sub, c0:SPAN],
                    kt[qrow : qrow + 64, i * 128 : (i + 1) * 128],
                    qt[qrow : qrow + 64, sp * SPAN + c0 : (sp + 1) * SPAN],
                    start=True,
                    stop=True,
                )
            nc.scalar.activation(
                pt[:, 2 * pair : 2 * pair + 2, c0:SPAN],
                ps[:, :, c0:SPAN],
                AF.Exp,
                scale=0.125,
            )
            if pair == 1 and d >= 0:
                # mask the triangular diagonal 128-col stripe for all 4 heads
                # (DVE; Pool tensor ops cost ~1.2us + library reloads)
                nc.vector.tensor_mul(
                    pt[:, :, c0 : c0 + 128],
                    pt[:, :, c0 : c0 + 128],
                    tri4[:].rearrange("p (h w) -> p h w", w=128),
                )

        def emit_pv_pair(i, pair):
            d = i - sp * SBS
            c0 = max(d, 0) * 128
            pt = pts[i] if pair == 0 else pts.pop(i)
            for sub in range(2):
                h = pair * 2 + sub
                if i == 0:
                    pos[h] = psum.tile(
                        [VW, SPAN], F32, name=f"po_{h}_{sp}", tag="acc", bufs=4
                    )
                nc.tensor.matmul(
                    pos[h][:, c0:SPAN],
                    vp[i][:, h * VW : (h + 1) * VW],
                    pt[:, h, c0:SPAN],
                    start=(i == 0),
                    stop=(i == nsb - 1),
                )
                if i == nsb - 1 and not is_last:
                    # den extraction on ACT: it is in its end-of-span lull
                    nc.scalar.copy(
                        den[32 * h : 32 * h + 1, :], pos[h][VW - 1 : VW, :]
                    )

        def emit_tail_quarter(q):
            # span-3 cols q*128:(q+1)*128 are final after PV k-block 12+q:
            # normalize + out-proj this quarter while later PV blocks run.
            # den hop via ACT (idle here), per-head recip at partition 0, no
            # rtmp: DVE only does recip + the normalize multiplies.
            cs = q * 128
            ce = cs + 128
            dts = []
            for h in range(HPC):
                dt = work.tile([1, 128], F32, name=f"dT_{h}_{q}", tag="dt", bufs=8)
                nc.scalar.copy(dt[:], pos[h][VW - 1 : VW, cs:ce])
                dts.append(dt)
            for h in range(HPC):
                dr = work.tile([1, 128], F32, name=f"drT_{h}_{q}", tag="dr", bufs=8)
                nc.vector.reciprocal_approx_fast(dr[:], dts[h][:])
                recb = work.tile([DH, 128], F32, name=f"rbT_{h}_{q}", tag="recb", bufs=2)
                nc.gpsimd.partition_broadcast(recb[:], dr[0:1, :])
                ot_tile = OT[(h * DH) // 128]
                orow = (h * DH) % 128
                nc.vector.tensor_mul(
                    ot_tile[orow : orow + DH, sp * SPAN + cs : sp * SPAN + ce],
                    pos[h][0:DH, cs:ce],
                    recb[:],
                )
            emit_outproj(
                12 + q,
                dma_engine=nc.scalar if q % 2 else nc.sync,
                split_dma=True,
            )

        for i in range(nsb + LAG):
            if i < nsb:
                emit_scores_pair(i, 0)
            if i >= LAG:
                emit_pv_pair(i - LAG, 0)
            if i < nsb:
                emit_scores_pair(i, 1)
            if i >= LAG:
                emit_pv_pair(i - LAG, 1)
            for fn in fills.get((sp, i), []):
                fn()
            if i >= LAG and is_last and i - LAG >= nsb - 4:
                emit_tail_quarter(i - LAG - (nsb - 4))

        return den, pos

    def emit_norm_recip(sp, den):
        denr = work.tile(
            [32 * (HPC - 1) + 1, SPAN], F32, name=f"denr_{sp}", tag="denr", bufs=1
        )
        nc.vector.reciprocal_approx_fast(denr[:], den[:])
        return denr

    def emit_norm_head(sp, h, denr, pos):
        # normalize straight out of the PV PSUM accumulator (no oraw copy)
        ot_tile = OT[(h * DH) // 128]
        orow = (h * DH) % 128
        rtmp = work.tile([1, SPAN], F32, name=f"rtmp_{h}_{sp}", tag="rtmp", bufs=2)
        nc.vector.tensor_copy(rtmp[:], denr[32 * h : 32 * h + 1, :])
        recb = work.tile([DH, SPAN], F32, name=f"recb_{h}_{sp}", tag="recb", bufs=2)
        nc.gpsimd.partition_broadcast(recb[:], rtmp[0:1, :])
        nc.vector.tensor_mul(
            ot_tile[orow : orow + DH, sp * SPAN : (sp + 1) * SPAN],
            pos[h][0:DH, :],
            recb[:],
        )

    # ---- flat emission: span-0 qk projections, then the attention stream ----
    for ob in range(NQK):
        emit_ph1_ob(0, ob)
    for sp in range(NSPAN):
        state[sp] = emit_attn(sp)


_NC_CACHE = {}


def _get_compiled():
    if "nc" not in _NC_CACHE:
        nc = bacc.Bacc(
            "TRN2", target_bir_lowering=False, debug=False, num_devices=N_CORES
        )
        io = _declare_io(nc)
        with tile.TileContext(nc) as tc, ExitStack() as ctx:
            _build(ctx, tc, io)
        nc.compile()
        _NC_CACHE["nc"] = nc
    return _NC_CACHE["nc"]


def _prep_core_inputs(x, W_qkv, b_qkv, W_out, b_out, core_id):
    g = core_id // CPG
    lane = core_id % CPG
    h0 = lane * HPC
    r = slice(h0 * DH, (h0 + HPC) * DH)
    Wq = W_qkv[0 * DM : 1 * DM, :][r, :]
    Wk = W_qkv[1 * DM : 2 * DM, :][r, :]
    Wv = W_qkv[2 * DM : 3 * DM, :][r, :]
    bq = b_qkv[0 * DM + h0 * DH : 0 * DM + (h0 + HPC) * DH]
    bk = b_qkv[1 * DM + h0 * DH : 1 * DM + (h0 + HPC) * DH]
    bv_ = b_qkv[2 * DM + h0 * DH : 2 * DM + (h0 + HPC) * DH]
    return {
        "xT": np.ascontiguousarray(x[g].T.astype(ml_dtypes.bfloat16)),
        "wqkT": np.ascontiguousarray(
            np.concatenate([Wq.T, Wk.T], axis=1).astype(ml_dtypes.bfloat16)
        ),
        "wvT": np.ascontiguousarray(Wv.T.astype(ml_dtypes.bfloat16)),
        "woT": np.ascontiguousarray(W_out[:, r].T.astype(ml_dtypes.bfloat16)),
        "bqk": np.concatenate([bq, bk]).reshape(2 * DQ, 1).astype(np.float32),
        "bv": np.ascontiguousarray(bv_.reshape(1, DQ).astype(np.float32)),
    }


def kernel(x, W_qkv, b_qkv, W_out, b_out, _trace=False):
    x = np.asarray(x)
    W_qkv = np.asarray(W_qkv)
    b_qkv = np.asarray(b_qkv)
    W_out = np.asarray(W_out)
    b_out = np.asarray(b_out)

    nc = _get_compiled()
    in_maps = [
        _prep_core_inputs(x, W_qkv, b_qkv, W_out, b_out, c) for c in range(N_CORES)
    ]
    res = run_bass_kernel_spmd(nc, in_maps, list(range(N_CORES)), trace=_trace)

    out = np.empty((B, S, DM), dtype=np.float32)
    for g in range(B):
        acc = res.results[g * CPG]["out"].astype(np.float32)
        for lane in range(1, CPG):
            acc = acc + res.results[g * CPG + lane]["out"]
        out[g] = acc + b_out[None, :].astype(np.float32)

    if _trace:
        kernel.last_exec_time_ns = res.exec_time_ns
        kernel.last_results = res
    return out


# revision 23
# speedup vs baseline: 1.3898x; 1.0068x over previous
"""Multi-head causal self-attention (torch nn.MultiheadAttention semantics)
on 8 Trainium2 NeuronCores.

Problem: x [2, 2048, 1024], 16 heads, head dim 64, fp32, causal, p_drop=0.

Sharding: 2 batch groups x 4-way head tensor-parallel.
  core c: batch b = c // 4, heads [lane*4, lane*4+4) with lane = c % 4.
Each core computes q/k/v projections for its 4 heads, flash-style causal
attention (S^T score layout, no-max softmax — scores are O(1) here), and its
partial out-projection. The host sums the 4 partials per batch and adds b_out.

v5 (trace-driven, from v3 @ 202us, v4b @ 188.5us):
  - Only sync (SP) and scalar (ACT) queues have fast hardware DMA (HWDGE,
    ~250GB/s multi-ring); gpsimd dma_start is a slow single-ring software
    path (~50GB/s). Critical prefix (wqk[0:2], x[0:2] of span 0, wv) issues
    on the scalar queue (ACT idle until first exp ~14us), the bulk on sync,
    only the tiny biases on gpsimd.
  - reciprocal -> reciprocal_approx_fast (~5x): the 3.35us iterative divide
    head-of-line blocked the PSUM-freeing bias-adds at span boundaries.
  - No oraw copies: normalize muls (DVE) read the PV PSUM accumulators
    directly (GPSIMD cannot touch PSUM; ACT copies were on the exp pacer).
  - W_out/OT bf16 (same PE rate at 512 free dim), b_v broadcast on chip.
  - LAG=3: PV trails scores more so PE rarely waits on the exp stream.
  - Fill chains spread evenly across steps (ACT exp pace is ~2.1us/step;
    clustered fills let the ps PSUM ring drain and stall later scores).
  - Tail: span-3 PV columns finalize progressively (cols q*128:(q+1)*128
    are final after k-block 12+q), so each 128-col quarter's den/normalize/
    out-proj/DMA chain is emitted right after that PV step, overlapping the
    remaining PV work; last out DMAs ride the fast scalar/sync queues.
"""

import os
from contextlib import ExitStack

import ml_dtypes
import numpy as np

import concourse.bass as bass
import concourse.tile as tile
from concourse import bacc, mybir
from concourse.bass_utils import run_bass_kernel_spmd

F32 = mybir.dt.float32
BF16 = mybir.dt.bfloat16
AF = mybir.ActivationFunctionType

B = 2
S = 2048
DM = 1024
N_HEADS = 16
DH = 64
N_CORES = 8
CPG = 4  # cores per group (tensor-parallel width over heads)
HPC = N_HEADS // CPG  # heads per core
DQ = HPC * DH
SPAN = 512
SB = 128
NDM = DM // 128
NSPAN = S // SPAN
NSB = S // SB
SBS = SPAN // SB
NQK = 2 * DQ // 128
NHD = DQ // 128
VW = DH + 1
OW = 512
NOUT = DM // OW
LAG = 3  # PV trails scores by this many sk blocks


def _declare_io(nc):
    t = {}
    t["xT"] = nc.dram_tensor("xT", [DM, S], BF16, kind="ExternalInput").ap()
    t["wqkT"] = nc.dram_tensor("wqkT", [DM, 2 * DQ], BF16, kind="ExternalInput").ap()
    t["wvT"] = nc.dram_tensor("wvT", [DM, DQ], BF16, kind="ExternalInput").ap()
    t["woT"] = nc.dram_tensor("woT", [DQ, DM], BF16, kind="ExternalInput").ap()
    t["bqk"] = nc.dram_tensor("bqk", [2 * DQ, 1], F32, kind="ExternalInput").ap()
    t["bv"] = nc.dram_tensor("bv", [1, DQ], F32, kind="ExternalInput").ap()
    t["out"] = nc.dram_tensor("out", [S, DM], BF16, kind="ExternalOutput").ap()
    return t


def _build(ctx: ExitStack, tc: tile.TileContext, io: dict):
    nc = tc.nc

    const = ctx.enter_context(tc.tile_pool(name="const", bufs=1))
    work = ctx.enter_context(tc.tile_pool(name="work", bufs=1))
    psum = ctx.enter_context(tc.tile_pool(name="psum", bufs=1, space="PSUM"))

    # ---- persistent tiles ----
    xTb = const.tile([128, NDM, S], BF16, name="xTb")
    wqkb = const.tile([128, NDM, 2 * DQ], BF16, name="wqkb")
    wvb = const.tile([128, NDM, DQ], BF16, name="wvb")
    wob = const.tile([128, NHD, DM], BF16, name="wob")
    bqkb = const.tile([128, NQK], F32, name="bqkb")
    bqk = [bqkb[:, b : b + 1] for b in range(NQK)]
    bvs = const.tile([1, DQ], F32, name="bvs")
    bv = const.tile([128, DQ], F32, name="bv")
    qkT = [const.tile([128, S], BF16, name=f"qkT{b}") for b in range(NQK)]
    vp = [const.tile([128, HPC * VW], BF16, name=f"vp{sb}") for sb in range(NSB)]
    OT = [const.tile([128, S], BF16, name=f"OT{c}") for c in range(NHD)]

    # ---- input DMAs (fast HWDGE queues: sync + scalar; gpsimd is slow).
    # Two balanced streams: first-needed piece leads each queue so the
    # ph1(0) chains can chase c-chunk arrivals. ----
    xTd = io["xT"].rearrange("(c p) s -> p c s", p=128)
    wqkd = io["wqkT"].rearrange("(c p) w -> p c w", p=128)
    for c in range(0, NDM, 2):
        nc.scalar.dma_start(wqkb[:, c : c + 2, :], wqkd[:, c : c + 2, :])
    nc.scalar.dma_start(wvb[:], io["wvT"].rearrange("(c p) w -> p c w", p=128))
    for c in range(0, NDM, 2):
        nc.sync.dma_start(xTb[:, c : c + 2, 0:SPAN], xTd[:, c : c + 2, 0:SPAN])
    for sp in range(1, NSPAN):
        nc.sync.dma_start(
            xTb[:, :, sp * SPAN : (sp + 1) * SPAN],
            xTd[:, :, sp * SPAN : (sp + 1) * SPAN],
        )
    nc.sync.dma_start(wob[:], io["woT"].rearrange("(c p) w -> p c w", p=128))
    nc.gpsimd.dma_start(bqkb[:], io["bqk"].rearrange("(b p) o -> p (b o)", p=128))
    nc.gpsimd.dma_start(bvs[:], io["bv"][:])

    # ---- constants: bv broadcast, vp ones-columns, replicated causal tri ----
    nc.gpsimd.partition_broadcast(bv[:], bvs[0:1, :])
    for sb in range(NSB):
        nc.gpsimd.memset(vp[sb][:, DH : HPC * VW : VW], 1.0)
    # tri4 = 4 side-by-side copies of tri[r, c] = (c - r >= 0)
    tri4 = const.tile([128, 4 * 128], BF16, name="tri4")
    nc.gpsimd.memset(tri4[:], 1.0)
    for k in range(4):
        nc.gpsimd.affine_select(
            out=tri4[:, k * 128 : (k + 1) * 128],
            in_=tri4[:, k * 128 : (k + 1) * 128],
            compare_op=mybir.AluOpType.is_ge,
            fill=0.0,
            base=0,
            pattern=[[1, 128]],
            channel_multiplier=-1,
        )

    # ---- PE warmup: dependency-free matmuls on a never-written tile run
    # during the input-DMA wait, ramping the PE pstate (0.65 -> 2.4 GHz
    # after ~3us continuous execution) so ph1(0) runs at full clock ----
    warm_src = const.tile([128, 640], BF16, name="warm_src")
    nc.vector.memset(warm_src[:], 0.0)
    for w in range(8):
        wp = psum.tile([128, SPAN], F32, name=f"warm{w}", tag="ps", bufs=2)
        nc.tensor.matmul(
            wp[:], warm_src[:, 0:128], warm_src[:, 128:640], start=True, stop=True
        )

    # ---- single-group emitters (fill work injected into attention steps) ----
    def emit_ph1_ob(sp, ob):
        pqk = psum.tile([128, SPAN], F32, name=f"pqk_{ob}_{sp}", tag="ps", bufs=2)
        for c in range(NDM):
            nc.tensor.matmul(
                pqk[:],
                wqkb[:, c, ob * 128 : (ob + 1) * 128],
                xTb[:, c, sp * SPAN : (sp + 1) * SPAN],
                start=(c == 0),
                stop=(c == NDM - 1),
            )
        nc.vector.tensor_scalar_add(
            qkT[ob][:, sp * SPAN : (sp + 1) * SPAN], pqk[:], bqk[ob][:]
        )

    def emit_vproj_sb(sb):
        pv = psum.tile([128, DQ], F32, name=f"pv_{sb}", tag="ps", bufs=2)
        for c in range(NDM):
            nc.tensor.matmul(
                pv[:],
                xTb[:, c, sb * 128 : (sb + 1) * 128],
                wvb[:, c, :],
                start=(c == 0),
                stop=(c == NDM - 1),
            )
        vdst = vp[sb][:, 0 : HPC * VW].rearrange("p (h w) -> p h w", w=VW)[:, :, 0:DH]
        nc.vector.tensor_add(
            vdst,
            pv[:].rearrange("p (h d) -> p h d", d=DH),
            bv[:].rearrange("p (h d) -> p h d", d=DH),
        )

    def emit_outproj(qb, dma_engine=None, split_dma=False, tail=False):
        ob_t = work.tile([128, DM], BF16, name=f"ob_{qb}", tag="ob", bufs=4)
        eng = dma_engine or nc.sync
        for nh in range(NOUT):
            pot = psum.tile([128, OW], F32, name=f"pot_{qb}_{nh}", tag="ps", bufs=2)
            for c in range(NHD):
                nc.tensor.matmul(
                    pot[:],
                    OT[c][:, qb * 128 : (qb + 1) * 128],
                    wob[:, c, nh * OW : (nh + 1) * OW],
                    start=(c == 0),
                    stop=(c == NHD - 1),
                )
            # mid-stream the exp stream saturates ACT (83%+ in spans 2-3)
            # while DVE idles, so fill blocks copy on DVE; at the tail ACT
            # is free again and the engines alternate
            if tail and nh % 2 == 0:
                nc.scalar.copy(ob_t[:, nh * OW : (nh + 1) * OW], pot[:])
            else:
                nc.vector.tensor_copy(ob_t[:, nh * OW : (nh + 1) * OW], pot[:])
            if split_dma:
                # each half leaves as soon as its PSUM copy lands (tail only:
                # shrinks the post-PE drain to the last 128KB transfer)
                eng.dma_start(
                    io["out"][qb * 128 : (qb + 1) * 128, nh * OW : (nh + 1) * OW],
                    ob_t[:, nh * OW : (nh + 1) * OW],
                )
        if not split_dma:
            eng.dma_start(io["out"][qb * 128 : (qb + 1) * 128, :], ob_t[:])

    # fill schedule: (sp, i) -> list of thunks. Spread so per-step PE work
    # roughly matches the ACT exp pace; late clusters starve the ps ring.
    fills = {}

    def add_fill(sp, i, fn):
        fills.setdefault((sp, i), []).append(fn)

    # normalize chain of span sp runs as fill in span sp+1's first steps.
    # Registered FIRST so the reciprocal leads the DVE queue at span start
    # (behind a projection-fill bias-add it stalls the acc-PSUM handoff).
    state = {}

    def norm_recip_fill(spp):
        def f():
            den, pos = state[spp]
            state[spp, "denr"] = emit_norm_recip(spp, den)

        return f

    def norm_head_fill(spp, h):
        def f():
            den, pos = state[spp]
            emit_norm_head(spp, h, state[spp, "denr"], pos)

        return f

    for spp in range(NSPAN - 1):
        add_fill(spp + 1, 0, norm_recip_fill(spp))
        add_fill(spp + 1, 0, norm_head_fill(spp, 0))
        add_fill(spp + 1, 0, norm_head_fill(spp, 1))
        add_fill(spp + 1, 1, norm_head_fill(spp, 2))
        add_fill(spp + 1, 1, norm_head_fill(spp, 3))

    # coverage: every step of every span carries ~one fill chain so the PE
    # never idles waiting on the ACT exp pace (idle also drops PE pstate)
    for j in range(4):
        add_fill(0, 1 + j, (lambda jj: lambda: emit_vproj_sb(jj))(j))
        add_fill(0, 3 + j, (lambda jj: lambda: emit_ph1_ob(1, jj))(j))
        add_fill(1, 2 * j, (lambda jj: lambda: emit_vproj_sb(4 + jj))(j))
        add_fill(1, 1 + 2 * j, (lambda jj: lambda: emit_ph1_ob(2, jj))(j))
        add_fill(2, 3 + 2 * j, (lambda jj: lambda: emit_ph1_ob(3, jj))(j))
        add_fill(2, 4 + 2 * j, (lambda jj: lambda: emit_vproj_sb(8 + jj))(j))
        add_fill(3, 2 + 2 * j, (lambda jj: lambda: emit_vproj_sb(12 + jj))(j))
    add_fill(1, 8, lambda: emit_outproj(0))
    add_fill(1, 9, lambda: emit_outproj(1))
    add_fill(1, 10, lambda: emit_outproj(2))
    add_fill(2, 2, lambda: emit_outproj(3))
    add_fill(2, 11, lambda: emit_outproj(4))
    add_fill(2, 12, lambda: emit_outproj(5))
    add_fill(2, 13, lambda: emit_outproj(6))
    add_fill(2, 14, lambda: emit_outproj(7))
    add_fill(3, 3, lambda: emit_outproj(8))
    add_fill(3, 10, lambda: emit_outproj(9))
    add_fill(3, 12, lambda: emit_outproj(10))
    add_fill(3, 14, lambda: emit_outproj(11))

    def emit_attn(sp):
        nsb = (sp + 1) * SBS
        is_last = sp == NSPAN - 1
        pos = {}
        pts = {}
        den = work.tile(
            [32 * (HPC - 1) + 1, SPAN], F32, name=f"den_{sp}", tag="den", bufs=1
        )

        def emit_scores_pair(i, pair):
            d = i - sp * SBS
            c0 = max(d, 0) * 128  # causal col offset within the span
            if pair == 0:
                pt = work.tile(
                    [128, HPC, SPAN], BF16, name=f"pt_{sp}_{i}", tag="pt", bufs=LAG + 2
                )
                pts[i] = pt
            pt = pts[i]
            ps = psum.tile(
                [128, 2, SPAN], F32, name=f"ps_{sp}_{i}_{pair}", tag="ps", bufs=2
            )
            for sub in range(2):
                h = pair * 2 + sub
                qt = qkT[h // 2]
                kt = qkT[NQK // 2 + h // 2]
                qrow = (h % 2) * 64
                nc.tensor.matmul(
                    ps[:, sub, c0:SPAN],
                    kt[qrow : qrow + 64, i * 128 : (i + 1) * 128],
                    qt[qrow : qrow + 64, sp * SPAN + c0 : (sp + 1) * SPAN],
                    start=True,
                    stop=True,
                )
            nc.scalar.activation(
                pt[:, 2 * pair : 2 * pair + 2, c0:SPAN],
                ps[:, :, c0:SPAN],
                AF.Exp,
                scale=0.125,
            )
            if pair == 1 and d >= 0:
                # mask the triangular diagonal 128-col stripe for all 4 heads
                # (DVE; Pool tensor ops cost ~1.2us + library reloads)
                nc.vector.tensor_mul(
                    pt[:, :, c0 : c0 + 128],
                    pt[:, :, c0 : c0 + 128],
                    tri4[:].rearrange("p (h w) -> p h w", w=128),
                )

        def emit_pv_pair(i, pair):
            d = i - sp * SBS
            c0 = max(d, 0) * 128
            pt = pts[i] if pair == 0 else pts.pop(i)
            for sub in range(2):
                h = pair * 2 + sub
                if i == 0:
                    pos[h] = psum.tile(
                        [VW, SPAN], F32, name=f"po_{h}_{sp}", tag="acc", bufs=4
                    )
                nc.tensor.matmul(
                    pos[h][:, c0:SPAN],
                    vp[i][:, h * VW : (h + 1) * VW],
                    pt[:, h, c0:SPAN],
                    start=(i == 0),
                    stop=(i == nsb - 1),
                )
                if i == nsb - 1 and not is_last:
                    # den extraction on ACT: it is in its end-of-span lull
                    nc.scalar.copy(
                        den[32 * h : 32 * h + 1, :], pos[h][VW - 1 : VW, :]
                    )

        def emit_tail_quarter(q):
            # span-3 cols q*128:(q+1)*128 are final after PV k-block 12+q:
            # normalize + out-proj this quarter while later PV blocks run.
            # den hop via ACT (idle here), per-head recip at partition 0, no
            # rtmp: DVE only does recip + the normalize multiplies.
            cs = q * 128
            ce = cs + 128
            dts = []
            for h in range(HPC):
                dt = work.tile([1, 128], F32, name=f"dT_{h}_{q}", tag="dt", bufs=8)
                nc.scalar.copy(dt[:], pos[h][VW - 1 : VW, cs:ce])
                dts.append(dt)
            for h in range(HPC):
                dr = work.tile([1, 128], F32, name=f"drT_{h}_{q}", tag="dr", bufs=8)
                nc.vector.reciprocal_approx_fast(dr[:], dts[h][:])
                recb = work.tile([DH, 128], F32, name=f"rbT_{h}_{q}", tag="recb", bufs=2)
                nc.gpsimd.partition_broadcast(recb[:], dr[0:1, :])
                ot_tile = OT[(h * DH) // 128]
                orow = (h * DH) % 128
                nc.vector.tensor_mul(
                    ot_tile[orow : orow + DH, sp * SPAN + cs : sp * SPAN + ce],
                    pos[h][0:DH, cs:ce],
                    recb[:],
                )
            emit_outproj(
                12 + q,
                dma_engine=nc.scalar if q % 2 else nc.sync,
                split_dma=True,
                tail=True,
            )

        for i in range(nsb + LAG):
            if i < nsb:
                emit_scores_pair(i, 0)
            if i >= LAG:
                emit_pv_pair(i - LAG, 0)
            if i < nsb:
                emit_scores_pair(i, 1)
            if i >= LAG:
                emit_pv_pair(i - LAG, 1)
            for fn in fills.get((sp, i), []):
                fn()
            if i >= LAG and is_last and i - LAG >= nsb - 4:
                emit_tail_quarter(i - LAG - (nsb - 4))

        return den, pos

    def emit_norm_recip(sp, den):
        denr = work.tile(
            [32 * (HPC - 1) + 1, SPAN], F32, name=f"denr_{sp}", tag="denr", bufs=1
        )
        nc.vector.reciprocal_approx_fast(denr[:], den[:])
        return denr

    def emit_norm_head(sp, h, denr, pos):
        # normalize straight out of the PV PSUM accumulator (no oraw copy)
        ot_tile = OT[(h * DH) // 128]
        orow = (h * DH) % 128
        rtmp = work.tile([1, SPAN], F32, name=f"rtmp_{h}_{sp}", tag="rtmp", bufs=2)
        nc.vector.tensor_copy(rtmp[:], denr[32 * h : 32 * h + 1, :])
        recb = work.tile([DH, SPAN], F32, name=f"recb_{h}_{sp}", tag="recb", bufs=2)
        nc.gpsimd.partition_broadcast(recb[:], rtmp[0:1, :])
        nc.vector.tensor_mul(
            ot_tile[orow : orow + DH, sp * SPAN : (sp + 1) * SPAN],
            pos[h][0:DH, :],
            recb[:],
        )

    # ---- flat emission: span-0 qk projections, then the attention stream ----
    for ob in range(NQK):
        emit_ph1_ob(0, ob)
    for sp in range(NSPAN):
        state[sp] = emit_attn(sp)


_NC_CACHE = {}


def _get_compiled():
    if "nc" not in _NC_CACHE:
        nc = bacc.Bacc(
            "TRN2", target_bir_lowering=False, debug=False, num_devices=N_CORES
        )
        io = _declare_io(nc)
        with tile.TileContext(nc) as tc, ExitStack() as ctx:
            _build(ctx, tc, io)
        nc.compile()
        _NC_CACHE["nc"] = nc
    return _NC_CACHE["nc"]


def _prep_core_inputs(x, W_qkv, b_qkv, W_out, b_out, core_id):
    g = core_id // CPG
    lane = core_id % CPG
    h0 = lane * HPC
    r = slice(h0 * DH, (h0 + HPC) * DH)
    Wq = W_qkv[0 * DM : 1 * DM, :][r, :]
    Wk = W_qkv[1 * DM : 2 * DM, :][r, :]
    Wv = W_qkv[2 * DM : 3 * DM, :][r, :]
    bq = b_qkv[0 * DM + h0 * DH : 0 * DM + (h0 + HPC) * DH]
    bk = b_qkv[1 * DM + h0 * DH : 1 * DM + (h0 + HPC) * DH]
    bv_ = b_qkv[2 * DM + h0 * DH : 2 * DM + (h0 + HPC) * DH]
    return {
        "xT": np.ascontiguousarray(x[g].T.astype(ml_dtypes.bfloat16)),
        "wqkT": np.ascontiguousarray(
            np.concatenate([Wq.T, Wk.T], axis=1).astype(ml_dtypes.bfloat16)
        ),
        "wvT": np.ascontiguousarray(Wv.T.astype(ml_dtypes.bfloat16)),
        "woT": np.ascontiguousarray(W_out[:, r].T.astype(ml_dtypes.bfloat16)),
        "bqk": np.concatenate([bq, bk]).reshape(2 * DQ, 1).astype(np.float32),
        "bv": np.ascontiguousarray(bv_.reshape(1, DQ).astype(np.float32)),
    }


def kernel(x, W_qkv, b_qkv, W_out, b_out, _trace=False):
    x = np.asarray(x)
    W_qkv = np.asarray(W_qkv)
    b_qkv = np.asarray(b_qkv)
    W_out = np.asarray(W_out)
    b_out = np.asarray(b_out)

    nc = _get_compiled()
    in_maps = [
        _prep_core_inputs(x, W_qkv, b_qkv, W_out, b_out, c) for c in range(N_CORES)
    ]
    res = run_bass_kernel_spmd(nc, in_maps, list(range(N_CORES)), trace=_trace)

    out = np.empty((B, S, DM), dtype=np.float32)
    for g in range(B):
        acc = res.results[g * CPG]["out"].astype(np.float32)
        for lane in range(1, CPG):
            acc = acc + res.results[g * CPG + lane]["out"]
        out[g] = acc + b_out[None, :].astype(np.float32)

    if _trace:
        kernel.last_exec_time_ns = res.exec_time_ns
        kernel.last_results = res
    return out
